# revision 11
# baseline (speedup 1.0000x reference)
"""Causal self-attention with dense global prefix, tensor-parallel over heads
across 8 Trainium2 NeuronCores.

Reference computation (T=4096, C=1024, H=16, D=64):
    qkv = x @ w_attn; q,k,v per head; scores = q k^T / sqrt(D)
    mask = causal | (col < num_frames); softmax; y = att @ v; out = y @ w_proj

Sharding: 2 heads per core. Each core computes its heads' attention output and
its slice of the output projection (w_proj rows for its heads), producing a
full-shape (T, C) partial; the host sums the 8 partials (the "all-reduce").

Device kernel layout choices:
  - x is transposed on the host once (xT: [C, T]) because the QKV matmul
    contracts over C, which must sit on SBUF partitions for both operands.
  - q, k are produced transposed ([D*2heads=128, T]) directly by the QKV
    matmul; scores are computed transposed (sT: [s, q]) so that the
    att @ v matmul consumes exp(sT) as the moving operand with no transposes.
  - v is produced transposed and flipped to natural [t, d] layout with PE
    transposes; a ones column is appended per head so the att @ v matmul also
    accumulates the softmax denominator (row 64 of its PSUM output).
  - softmax skips the max-subtraction: scores are ~N(0,1) after the 1/8
    scale, so exp never overflows fp32.
  - matmuls run in float32r (TF32-like, 1 cycle/row) via bitcast views.
  - the prefix+causal mask is applied multiplicatively to exp(scores) on the
    diagonal blocks only; mask tiles are built host-side for the actual
    num_frames value at trace time.
"""

import sys

if "/opt/trn_rl_repo" not in sys.path:
    sys.path.insert(0, "/opt/trn_rl_repo")

import numpy as np

import concourse.bacc as bacc
import concourse.mybir as mybir
from concourse.tile import TileContext
from concourse import bass_utils

T = 4096
C = 1024
H = 16
D = 64
NCORES = 8
HPC = H // NCORES          # heads per core = 2
QC = 512                   # q-chunk (moving free dim)
NQ = T // QC               # 8 q-chunks
NCH = C // 128             # 8 contraction chunks for QKV
NT = T // 128              # 32 t-tiles
F32 = mybir.dt.float32
F32R = mybir.dt.float32r

_cache = {}


def _mask_tiles(nf: int):
    """Mask tiles for diagonal score blocks, deduped.

    In sT layout a tile covers s in [128*st, 128*st+128) (partitions) and
    q in [512*j, 512*j+512) (free).  Entry (s, q) is visible iff s <= q or
    s < nf.  A tile needs masking iff st >= 4j (diagonal) and not fully
    visible.  Pattern key: (m, pr) with m = st - 4j, pr = rows fully visible
    from the prefix.
    """
    tiles = {}       # (m, pr) -> index
    arrs = []
    idx_map = {}     # (j, st) -> index or None (no mask needed)
    p = np.arange(128)[:, None]
    q = np.arange(QC)[None, :]
    for j in range(NQ):
        for st in range(4 * j, 4 * j + 4):
            s0 = 128 * st
            pr = int(np.clip(nf - s0, 0, 128))
            m = st - 4 * j
            causal = (s0 + p) <= (512 * j + q)
            vis = causal | ((s0 + p) < nf)
            if vis.all():
                idx_map[(j, st)] = None
                continue
            key = (m, pr)
            if key not in tiles:
                tiles[key] = len(arrs)
                arrs.append(vis.astype(np.float32))
            idx_map[(j, st)] = tiles[key]
    if not arrs:  # degenerate: everything visible
        arrs.append(np.ones((128, QC), np.float32))
    return np.stack(arrs), idx_map


def _build(nf: int, n_masks: int):
    nc = bacc.Bacc("TRN2", target_bir_lowering=False)

    xT_d = nc.dram_tensor("xT", [C, T], F32R, kind="ExternalInput")
    wqkv_d = nc.dram_tensor("wqkv", [C, 3 * 128], F32R, kind="ExternalInput")
    wp_d = nc.dram_tensor("wp", [128, C], F32R, kind="ExternalInput")
    masks_d = nc.dram_tensor("masks", [n_masks, 128, QC], F32R, kind="ExternalInput")
    ident_d = nc.dram_tensor("ident", [128, 128], F32, kind="ExternalInput")
    ones2_d = nc.dram_tensor("ones2", [1, 256], F32R, kind="ExternalInput")
    vones_d = nc.dram_tensor("vones", [128, NT], F32R, kind="ExternalInput")
    y_d = nc.dram_tensor("y_part", [T, C], F32, kind="ExternalOutput")

    _, idx_map = _mask_tiles(nf)

    with TileContext(nc) as tc:
        with tc.tile_pool(name="persist", bufs=1) as pp:
            qT = pp.tile([128, T], F32R, tag="qT")
            kT = pp.tile([128, T], F32R, tag="kT")
            # v in natural [t, d] layout, 130 cols per t-tile:
            # [v_h0 (64) | ones | v_h1 (64) | ones]
            vsb = pp.tile([128, NT, 130], F32R, tag="vsb")
            wqkv = pp.tile([128, NCH, 3 * 128], F32R, tag="wqkv")
            wp = pp.tile([128, C], F32R, tag="wp")
            msk = pp.tile([128, n_masks, QC], F32R, tag="msk")
            ident = pp.tile([128, 128], F32, tag="ident")
            ones2 = pp.tile([1, 256], F32R, tag="ones2")
            vones = pp.tile([128, NT], F32R, tag="vones")

            nc.sync.dma_start(out=wqkv[:, :, :], in_=wqkv_d.ap().rearrange("(n p) m -> p n m", p=128))
            nc.sync.dma_start(out=wp[:, :], in_=wp_d[:, :])
            nc.sync.dma_start(out=msk[:, :, :], in_=masks_d.ap().rearrange("n p q -> p n q"))
            nc.sync.dma_start(out=ident[:, :], in_=ident_d[:, :])
            nc.sync.dma_start(out=ones2[:, :], in_=ones2_d[:, :])
            nc.sync.dma_start(out=vones[:, :], in_=vones_d[:, :])
            nc.vector.tensor_copy(vsb[:, :, 64:65], vones[:, :])
            nc.vector.tensor_copy(vsb[:, :, 129:130], vones[:, :])

            xT_r = xT_d.ap().rearrange("(n p) t -> p n t", p=128)

            # ---------------- Phase 1: QKV projection ----------------
            with tc.tile_pool(name="p1sb", bufs=2) as p1sb, \
                 tc.tile_pool(name="p1ps", bufs=3, space="PSUM") as p1ps, \
                 tc.tile_pool(name="p1tr", bufs=2, space="PSUM") as p1tr:
                for i in range(NQ):  # 8 chunks of 512 t-columns
                    t0 = i * QC
                    xt = p1sb.tile([128, NCH, QC], F32R, tag="xt")
                    nc.sync.dma_start(out=xt[:, :, :], in_=xT_r[:, :, t0:t0 + QC])
                    for m in range(3):  # q, k, v
                        pm = p1ps.tile([128, QC], F32, tag="pm")
                        for n in range(NCH):
                            nc.tensor.matmul(
                                pm[:, :],
                                wqkv[:, n, 128 * m:128 * (m + 1)],
                                xt[:, n, :],
                                start=(n == 0), stop=(n == NCH - 1),
                            )
                        if m == 0:
                            nc.vector.tensor_copy(qT[:, t0:t0 + QC], pm[:, :])
                        elif m == 1:
                            nc.vector.tensor_copy(kT[:, t0:t0 + QC], pm[:, :])
                        else:
                            vstage = p1sb.tile([128, QC], F32, tag="vstage")
                            nc.vector.tensor_copy(vstage[:, :], pm[:, :])
                            for k4 in range(4):
                                tt = 4 * i + k4
                                vtp = p1tr.tile([128, 128], F32, tag="vtp")
                                nc.tensor.transpose(
                                    vtp[:, :], vstage[:, 128 * k4:128 * (k4 + 1)],
                                    ident[:, :])
                                nc.vector.tensor_copy(vsb[:, tt, 0:64], vtp[:, 0:64])
                                nc.vector.tensor_copy(vsb[:, tt, 65:129], vtp[:, 64:128])

            # ---------------- Phase 2: attention + projection ----------------
            with tc.tile_pool(name="ag", bufs=2, space="PSUM") as agp, \
                 tc.tile_pool(name="yt", bufs=2, space="PSUM") as ytp, \
                 tc.tile_pool(name="po", bufs=1, space="PSUM") as pop, \
                 tc.tile_pool(name="esb", bufs=4) as esb, \
                 tc.tile_pool(name="nsb", bufs=2) as nsb:
                for j in range(NQ):
                    q0 = j * QC
                    nst = 4 * j + 4
                    yt = [ytp.tile([128, QC], F32, tag="yt", name=f"yt{j}_{h}")
                          for h in range(HPC)]
                    for g in range(nst // 2):
                        for h in range(HPC):
                            ag = agp.tile([128, 1024], F32, tag="ag")
                            for u in range(2):
                                st = 2 * g + u
                                nc.tensor.matmul(
                                    ag[:, QC * u:QC * (u + 1)],
                                    kT[64 * h:64 * h + 64, 128 * st:128 * (st + 1)],
                                    qT[64 * h:64 * h + 64, q0:q0 + QC],
                                    start=True, stop=True,
                                )
                            ex = esb.tile([128, 1024], F32R, tag="ex")
                            nc.scalar.activation(
                                ex[:, :], ag[:, :],
                                mybir.ActivationFunctionType.Exp, scale=0.125)
                            for u in range(2):
                                st = 2 * g + u
                                mi = idx_map[(j, st)] if st >= 4 * j else None
                                if mi is not None:
                                    nc.vector.tensor_mul(
                                        ex[:, QC * u:QC * (u + 1)],
                                        ex[:, QC * u:QC * (u + 1)],
                                        msk[:, mi, :])
                            for u in range(2):
                                st = 2 * g + u
                                nc.tensor.matmul(
                                    yt[h][0:65, :],
                                    vsb[:, st, 65 * h:65 * h + 65],
                                    ex[:, QC * u:QC * (u + 1)],
                                    start=(st == 0), stop=(st == nst - 1),
                                    skip_group_check=True,
                                )
                    # normalization: rec = 1/denom, broadcast to head rows
                    rec = [nsb.tile([1, QC], F32R, tag=f"rec{h}", name=f"rec{j}_{h}")
                           for h in range(HPC)]
                    with nc.allow_low_precision(reason="f32r holds full-precision reciprocal bits"):
                        for h in range(HPC):
                            nc.vector.reciprocal(rec[h][0:1, :], yt[h][64:65, :])
                    rb = agp.tile([128, QC], F32, tag="ag")
                    for h in range(HPC):
                        nc.tensor.matmul(rb[:, :],
                                         ones2[0:1, 128 * h:128 * (h + 1)],
                                         rec[h][0:1, :],
                                         start=(h == 0), stop=(h == HPC - 1),
                                         skip_group_check=True)
                    rbs = nsb.tile([128, QC], F32, tag="rbs")
                    nc.vector.tensor_copy(rbs[:, :], rb[:, :])
                    yn = nsb.tile([128, QC], F32R, tag="yn")
                    for h in range(HPC):
                        nc.vector.tensor_mul(
                            yn[64 * h:64 * h + 64, :],
                            yt[h][0:64, :],
                            rbs[64 * h:64 * h + 64, :])
                    # output projection for this q-chunk
                    for k4 in range(4):
                        po = pop.tile([128, 1024], F32, tag="po")
                        for co in range(2):
                            nc.tensor.matmul(
                                po[:, QC * co:QC * (co + 1)],
                                yn[:, 128 * k4:128 * (k4 + 1)],
                                wp[:, QC * co:QC * (co + 1)],
                                start=True, stop=True,
                            )
                        posb = nsb.tile([128, 1024], F32, tag="posb")
                        nc.vector.tensor_copy(posb[:, :], po[:, :])
                        nc.sync.dma_start(
                            out=y_d[q0 + 128 * k4:q0 + 128 * (k4 + 1), :],
                            in_=posb[:, :])

    nc.compile()
    return nc


class _Runner:
    """Compile once; execute the SPMD NEFF via PJRT shard_map.

    Mirrors bass2jax.run_bass_via_pjrt's multi-core branch, but without
    donating the output buffers so the jitted callable can be re-invoked on
    device-resident inputs (for timing) without re-uploading zeros.
    """

    def __init__(self, nc):
        import jax
        import concourse.mybir as _mybir
        from jax.experimental.shard_map import shard_map
        from jax.sharding import Mesh, PartitionSpec
        from concourse.bass2jax import (_bass_exec_p, install_neuronx_cc_hook,
                                        partition_id_tensor)

        install_neuronx_cc_hook()
        self.nc = nc
        partition_name = nc.partition_id_tensor.name if nc.partition_id_tensor else None
        in_names, out_names, out_avals = [], [], []
        for alloc in nc.m.functions[0].allocations:
            if not isinstance(alloc, _mybir.MemoryLocationSet):
                continue
            name = alloc.memorylocations[0].name
            if alloc.kind == "ExternalInput":
                if name != partition_name:
                    in_names.append(name)
            elif alloc.kind == "ExternalOutput":
                out_names.append(name)
                out_avals.append(jax.core.ShapedArray(
                    tuple(alloc.tensor_shape), _mybir.dt.np(alloc.dtype)))
        self.in_names = list(in_names)
        self.out_names = out_names
        self.out_avals = out_avals
        n_params = len(in_names)
        all_in_names = in_names + out_names
        if partition_name is not None:
            all_in_names.append(partition_name)

        def _body(*args):
            operands = list(args)
            if partition_name is not None:
                operands.append(partition_id_tensor())
            return tuple(_bass_exec_p.bind(
                *operands,
                out_avals=tuple(out_avals),
                in_names=tuple(all_in_names),
                out_names=tuple(out_names),
                lowering_input_output_aliases=(),
                sim_require_finite=True,
                sim_require_nnan=True,
                nc=nc,
            ))

        devices = jax.devices()[:NCORES]
        self.mesh = Mesh(np.asarray(devices), ("core",))
        nin = n_params + len(out_names)
        self.fn = jax.jit(shard_map(
            _body, mesh=self.mesh,
            in_specs=(PartitionSpec("core"),) * nin,
            out_specs=(PartitionSpec("core"),) * len(out_names),
            check_rep=False), keep_unused=True)
        self._zeros = None

    def device_inputs(self, in_maps):
        import jax
        concat = [np.concatenate([np.asarray(m[n]) for m in in_maps], axis=0)
                  for n in self.in_names]
        if self._zeros is None:
            self._zeros = [
                jax.device_put(np.zeros((NCORES * a.shape[0], *a.shape[1:]), a.dtype))
                for a in self.out_avals]
        return [jax.device_put(c) for c in concat] + self._zeros

    def run(self, dev_inputs):
        outs = self.fn(*dev_inputs)
        return outs

    def gather(self, outs):
        res = []
        for c in range(NCORES):
            res.append({
                name: np.asarray(outs[i]).reshape(NCORES, *self.out_avals[i].shape)[c]
                for i, name in enumerate(self.out_names)})
        return res


def get_runner(num_frames=64):
    nf = int(np.asarray(num_frames))
    masks, _ = _mask_tiles(nf)
    key = (nf, masks.shape[0])
    if key not in _cache:
        _cache[key] = _Runner(_build(nf, masks.shape[0]))
    return _cache[key], masks


def make_in_maps(x, w_attn, w_proj, masks):

    xT = np.ascontiguousarray(x.T)
    ident = np.eye(128, dtype=np.float32)
    ones2 = np.zeros((1, 256), np.float32)
    ones2[0, 0:64] = 1.0
    ones2[0, 192:256] = 1.0

    in_maps = []
    for c in range(NCORES):
        h0, h1 = HPC * c, HPC * c + 1
        wq = np.concatenate([w_attn[:, D * h0:D * h0 + D],
                             w_attn[:, D * h1:D * h1 + D]], axis=1)
        wk = np.concatenate([w_attn[:, C + D * h0:C + D * h0 + D],
                             w_attn[:, C + D * h1:C + D * h1 + D]], axis=1)
        wv = np.concatenate([w_attn[:, 2 * C + D * h0:2 * C + D * h0 + D],
                             w_attn[:, 2 * C + D * h1:2 * C + D * h1 + D]], axis=1)
        wqkv = np.ascontiguousarray(np.concatenate([wq, wk, wv], axis=1))
        wp = np.ascontiguousarray(
            np.concatenate([w_proj[D * h0:D * h0 + D, :],
                            w_proj[D * h1:D * h1 + D, :]], axis=0))
        in_maps.append({
            "xT": xT, "wqkv": wqkv, "wp": wp,
            "masks": masks, "ident": ident, "ones2": ones2,
            "vones": np.ones((128, NT), np.float32),
        })
    return in_maps


def kernel(x, w_attn, w_proj, num_frames):
    x = np.asarray(x, dtype=np.float32)
    w_attn = np.asarray(w_attn, dtype=np.float32)
    w_proj = np.asarray(w_proj, dtype=np.float32)

    runner, masks = get_runner(num_frames)
    in_maps = make_in_maps(x, w_attn, w_proj, masks)
    outs = runner.run(runner.device_inputs(in_maps))
    results = runner.gather(outs)
    acc = np.zeros((T, C), np.float64)
    for rmap in results:
        acc += rmap["y_part"].astype(np.float64)
    return acc.astype(np.float32)


# revision 15
# speedup vs baseline: 1.2454x; 1.2454x over previous
"""Causal self-attention with dense global prefix, tensor-parallel over heads
across 8 Trainium2 NeuronCores.

Reference computation (T=4096, C=1024, H=16, D=64):
    qkv = x @ w_attn; q,k,v per head; scores = q k^T / sqrt(D)
    mask = causal | (col < num_frames); softmax; y = att @ v; out = y @ w_proj

Sharding: 2 heads per core. Each core computes its heads' attention output and
its slice of the output projection (w_proj rows for its heads), producing a
full-shape (T, C) partial; the host sums the 8 partials (the "all-reduce").

Device kernel layout choices:
  - x is transposed on the host once (xT: [C, T]) because the QKV matmul
    contracts over C, which must sit on SBUF partitions for both operands.
  - q, k are produced transposed ([D*2heads=128, T]) directly by the QKV
    matmul; scores are computed transposed (sT: [s, q]) so that the
    att @ v matmul consumes exp(sT) as the moving operand with no transposes.
  - v is produced transposed and flipped to natural [t, d] layout with PE
    transposes; a ones column is appended per head so the att @ v matmul also
    accumulates the softmax denominator (row 64 of its PSUM output).
  - softmax skips the max-subtraction: scores are ~N(0,1) after the 1/8
    scale, so exp never overflows fp32.
  - matmuls run in float32r (TF32-like, 1 cycle/row) via bitcast views.
  - the prefix+causal mask is applied multiplicatively to exp(scores) on the
    diagonal blocks only; mask tiles are built host-side for the actual
    num_frames value at trace time.
"""

import sys

if "/opt/trn_rl_repo" not in sys.path:
    sys.path.insert(0, "/opt/trn_rl_repo")

import numpy as np

import concourse.bacc as bacc
import concourse.mybir as mybir
from concourse.tile import TileContext
from concourse import bass_utils

T = 4096
C = 1024
H = 16
D = 64
NCORES = 8
HPC = H // NCORES          # heads per core = 2
QC = 512                   # q-chunk (moving free dim)
NQ = T // QC               # 8 q-chunks
NCH = C // 128             # 8 contraction chunks for QKV
NT = T // 128              # 32 t-tiles
F32 = mybir.dt.float32
F32R = mybir.dt.float32r

_cache = {}


def _mask_tiles(nf: int):
    """Mask tiles for diagonal score blocks, deduped.

    In sT layout a tile covers s in [128*st, 128*st+128) (partitions) and
    q in [512*j, 512*j+512) (free).  Entry (s, q) is visible iff s <= q or
    s < nf.  A tile needs masking iff st >= 4j (diagonal) and not fully
    visible.  Pattern key: (m, pr) with m = st - 4j, pr = rows fully visible
    from the prefix.
    """
    tiles = {}       # (m, pr) -> index
    arrs = []
    idx_map = {}     # (j, st) -> index or None (no mask needed)
    p = np.arange(128)[:, None]
    q = np.arange(QC)[None, :]
    for j in range(NQ):
        for st in range(4 * j, 4 * j + 4):
            s0 = 128 * st
            pr = int(np.clip(nf - s0, 0, 128))
            m = st - 4 * j
            causal = (s0 + p) <= (512 * j + q)
            vis = causal | ((s0 + p) < nf)
            if vis.all():
                idx_map[(j, st)] = None
                continue
            key = (m, pr)
            if key not in tiles:
                tiles[key] = len(arrs)
                arrs.append(vis.astype(np.float32))
            idx_map[(j, st)] = tiles[key]
    if not arrs:  # degenerate: everything visible
        arrs.append(np.ones((128, QC), np.float32))
    return np.stack(arrs), idx_map


def _build(nf: int, n_masks: int):
    nc = bacc.Bacc("TRN2", target_bir_lowering=False)

    xT_d = nc.dram_tensor("xT", [C, T], F32R, kind="ExternalInput")
    wqkv_d = nc.dram_tensor("wqkv", [C, 3 * 128], F32R, kind="ExternalInput")
    wp_d = nc.dram_tensor("wp", [128, C], F32R, kind="ExternalInput")
    masks_d = nc.dram_tensor("masks", [n_masks, 128, QC], F32R, kind="ExternalInput")
    # aux: cols 0-127 identity, 128-159 vones, row0 cols 160-415 bcast patterns
    aux_d = nc.dram_tensor("aux", [128, 416], F32R, kind="ExternalInput")
    y_d = nc.dram_tensor("y_part", [T, C], F32, kind="ExternalOutput")

    _, idx_map = _mask_tiles(nf)
    import os
    MASK_ENGINE = nc.gpsimd if os.environ.get("MASK_ENGINE", "dve") == "pool" else nc.vector

    with TileContext(nc) as tc:
        with tc.tile_pool(name="persist", bufs=1) as pp, \
             tc.tile_pool(name="xsb", bufs=2) as xsb, \
             tc.tile_pool(name="agp", bufs=3, space="PSUM") as agp, \
             tc.tile_pool(name="ytp", bufs=2, space="PSUM") as ytp, \
             tc.tile_pool(name="esb", bufs=4) as esb, \
             tc.tile_pool(name="nsb", bufs=2) as nsb, \
             tc.tile_pool(name="pob", bufs=2) as pob:
            # per-chunk tensors (separate tiles -> no false WAR deps between
            # later QKV writes and earlier attention reads)
            qTc = [pp.tile([128, QC], F32R, tag=f"qT{i}", name=f"qT{i}") for i in range(NQ)]
            kTc = [pp.tile([128, QC], F32R, tag=f"kT{i}", name=f"kT{i}") for i in range(NQ)]
            # v natural layout per chunk: 4 t-tiles x [v_h0 | ones | v_h1 | ones]
            vsbc = [pp.tile([128, 4, 130], F32R, tag=f"vsb{i}", name=f"vsb{i}") for i in range(NQ)]
            wqkv = pp.tile([128, NCH, 3 * 128], F32R, tag="wqkv")
            wp = pp.tile([128, C], F32R, tag="wp")
            msk = pp.tile([128, n_masks, QC], F32R, tag="msk")
            aux = pp.tile([128, 416], F32R, tag="aux")
            ident = aux[:, 0:128]
            vones = aux[:, 128:160]
            ones2 = aux[0:1, 160:416]

            nc.sync.dma_start(out=wqkv[:, :, :], in_=wqkv_d.ap().rearrange("(n p) m -> p n m", p=128))
            nc.sync.dma_start(out=wp[:, :], in_=wp_d[:, :])
            nc.sync.dma_start(out=msk[:, :, :], in_=masks_d.ap().rearrange("n p q -> p n q"))
            nc.sync.dma_start(out=aux[:, :], in_=aux_d[:, :])
            for i in range(NQ):
                nc.vector.tensor_copy(vsbc[i][:, :, 64:65], vones[:, 4 * i:4 * i + 4])
                nc.vector.tensor_copy(vsbc[i][:, :, 129:130], vones[:, 4 * i:4 * i + 4])

            xT_r = xT_d.ap().rearrange("(n p) t -> p n t", p=128)

            for j in range(NQ):   # combined QKV(j) + attention(j)
                t0 = j * QC
                # ---- QKV for t-chunk j ----
                xt = xsb.tile([128, NCH, QC], F32R, tag="xt")
                nc.sync.dma_start(out=xt[:, :, :], in_=xT_r[:, :, t0:t0 + QC])
                for m in range(3):  # q, k, v
                    pm = agp.tile([128, QC], F32, tag="agp", name=f"pm{j}_{m}")
                    for n in range(NCH):
                        nc.tensor.matmul(
                            pm[:, :],
                            wqkv[:, n, 128 * m:128 * (m + 1)],
                            xt[:, n, :],
                            start=(n == 0), stop=(n == NCH - 1),
                        )
                    if m == 0:
                        nc.vector.tensor_copy(qTc[j][:, :], pm[:, :])
                    elif m == 1:
                        nc.vector.tensor_copy(kTc[j][:, :], pm[:, :])
                    else:
                        vstage = xsb.tile([128, QC], F32R, tag="vstage")
                        nc.vector.tensor_copy(vstage[:, :], pm[:, :])
                        vtp4 = agp.tile([128, QC], F32, tag="agp", name=f"vtp{j}")
                        for k4 in range(4):
                            nc.tensor.transpose(
                                vtp4[:, 128 * k4:128 * (k4 + 1)].bitcast(F32R),
                                vstage[:, 128 * k4:128 * (k4 + 1)],
                                ident)
                        nc.vector.tensor_copy(
                            vsbc[j][:, :, :]
                                .rearrange("p t (h c) -> p t h c", h=2)[:, :, :, 0:64],
                            vtp4[:, :].rearrange("p (t h c) -> p t h c", t=4, h=2))

                # ---- attention for q-chunk j ----
                q0 = t0
                nst = 4 * j + 4
                yt = [ytp.tile([128, QC], F32, tag="yt", name=f"yt{j}_{h}")
                      for h in range(HPC)]
                # diagonal (masked) groups first so their mask-muls overlap
                # later groups' matmuls instead of sitting on the tail
                glist = list(range(nst // 2))[::-1]
                nb = [0, 0]
                for g in glist:
                    for h in range(HPC):
                        ag = agp.tile([128, 1024], F32, tag="agp", name=f"ag{j}_{g}_{h}")
                        for u in range(2):
                            st = 2 * g + u
                            nc.tensor.matmul(
                                ag[:, QC * u:QC * (u + 1)],
                                kTc[st // 4][64 * h:64 * h + 64, 128 * (st % 4):128 * (st % 4 + 1)],
                                qTc[j][64 * h:64 * h + 64, :],
                                start=True, stop=True,
                            )
                        ex = esb.tile([128, 1024], F32R, tag="ex")
                        nc.scalar.activation(
                            ex[:, :], ag[:, :],
                            mybir.ActivationFunctionType.Exp, scale=0.125)
                        for u in range(2):
                            st = 2 * g + u
                            mi = idx_map[(j, st)] if st >= 4 * j else None
                            if mi is not None:
                                MASK_ENGINE.tensor_mul(
                                    ex[:, QC * u:QC * (u + 1)],
                                    ex[:, QC * u:QC * (u + 1)],
                                    msk[:, mi, :])
                        for u in range(2):
                            st = 2 * g + u
                            nc.tensor.matmul(
                                yt[h][0:65, :],
                                vsbc[st // 4][:, st % 4, 65 * h:65 * h + 65],
                                ex[:, QC * u:QC * (u + 1)],
                                start=(nb[h] == 0), stop=(nb[h] == nst - 1),
                                skip_group_check=True,
                            )
                            nb[h] += 1
                # normalization: rec = 1/denom, broadcast to head rows
                rec = [nsb.tile([1, QC], F32R, tag=f"rec{h}", name=f"rec{j}_{h}")
                       for h in range(HPC)]
                with nc.allow_low_precision(reason="f32r holds full-precision reciprocal bits"):
                    for h in range(HPC):
                        nc.vector.reciprocal(rec[h][0:1, :], yt[h][64:65, :])
                rb = agp.tile([128, QC], F32, tag="agp", name=f"rb{j}")
                for h in range(HPC):
                    nc.tensor.matmul(rb[:, :],
                                     ones2[0:1, 128 * h:128 * (h + 1)],
                                     rec[h][0:1, :],
                                     start=(h == 0), stop=(h == HPC - 1),
                                     skip_group_check=True)
                rbs = nsb.tile([128, QC], F32, tag="rbs")
                nc.vector.tensor_copy(rbs[:, :], rb[:, :])
                yn = nsb.tile([128, QC], F32R, tag="yn")
                for h in range(HPC):
                    nc.vector.tensor_mul(
                        yn[64 * h:64 * h + 64, :],
                        yt[h][0:64, :],
                        rbs[64 * h:64 * h + 64, :])
                # output projection for this q-chunk, staged then one DMA
                posb = pob.tile([128, 4, 1024], F32, tag="posb")
                for k4 in range(4):
                    po = agp.tile([128, 1024], F32, tag="agp", name=f"po{j}_{k4}")
                    for co in range(2):
                        nc.tensor.matmul(
                            po[:, QC * co:QC * (co + 1)],
                            yn[:, 128 * k4:128 * (k4 + 1)],
                            wp[:, QC * co:QC * (co + 1)],
                            start=True, stop=True,
                        )
                    nc.vector.tensor_copy(posb[:, k4, :], po[:, :])
                nc.sync.dma_start(
                    out=y_d[q0:q0 + QC, :].rearrange("(k p) c -> p k c", p=128),
                    in_=posb[:, :, :])

    nc.compile()
    return nc


class _Runner:
    """Compile once; execute the SPMD NEFF via PJRT shard_map.

    Mirrors bass2jax.run_bass_via_pjrt's multi-core branch, but without
    donating the output buffers so the jitted callable can be re-invoked on
    device-resident inputs (for timing) without re-uploading zeros.
    """

    def __init__(self, nc):
        import jax
        import concourse.mybir as _mybir
        from jax.experimental.shard_map import shard_map
        from jax.sharding import Mesh, PartitionSpec
        from concourse.bass2jax import (_bass_exec_p, install_neuronx_cc_hook,
                                        partition_id_tensor)

        install_neuronx_cc_hook()
        self.nc = nc
        partition_name = nc.partition_id_tensor.name if nc.partition_id_tensor else None
        in_names, out_names, out_avals = [], [], []
        for alloc in nc.m.functions[0].allocations:
            if not isinstance(alloc, _mybir.MemoryLocationSet):
                continue
            name = alloc.memorylocations[0].name
            if alloc.kind == "ExternalInput":
                if name != partition_name:
                    in_names.append(name)
            elif alloc.kind == "ExternalOutput":
                out_names.append(name)
                out_avals.append(jax.core.ShapedArray(
                    tuple(alloc.tensor_shape), _mybir.dt.np(alloc.dtype)))
        self.in_names = list(in_names)
        self.out_names = out_names
        self.out_avals = out_avals
        n_params = len(in_names)
        all_in_names = in_names + out_names
        if partition_name is not None:
            all_in_names.append(partition_name)

        def _body(*args):
            operands = list(args)
            if partition_name is not None:
                operands.append(partition_id_tensor())
            return tuple(_bass_exec_p.bind(
                *operands,
                out_avals=tuple(out_avals),
                in_names=tuple(all_in_names),
                out_names=tuple(out_names),
                lowering_input_output_aliases=(),
                sim_require_finite=True,
                sim_require_nnan=True,
                nc=nc,
            ))

        devices = jax.devices()[:NCORES]
        self.mesh = Mesh(np.asarray(devices), ("core",))
        nin = n_params + len(out_names)
        self.fn = jax.jit(shard_map(
            _body, mesh=self.mesh,
            in_specs=(PartitionSpec("core"),) * nin,
            out_specs=(PartitionSpec("core"),) * len(out_names),
            check_rep=False), keep_unused=True)
        self._zeros = None

    def device_inputs(self, in_maps):
        import jax
        concat = [np.concatenate([np.asarray(m[n]) for m in in_maps], axis=0)
                  for n in self.in_names]
        if self._zeros is None:
            self._zeros = [
                jax.device_put(np.zeros((NCORES * a.shape[0], *a.shape[1:]), a.dtype))
                for a in self.out_avals]
        return [jax.device_put(c) for c in concat] + self._zeros

    def run(self, dev_inputs):
        outs = self.fn(*dev_inputs)
        return outs

    def gather(self, outs):
        res = []
        for c in range(NCORES):
            res.append({
                name: np.asarray(outs[i]).reshape(NCORES, *self.out_avals[i].shape)[c]
                for i, name in enumerate(self.out_names)})
        return res


def get_runner(num_frames=64):
    nf = int(np.asarray(num_frames))
    masks, _ = _mask_tiles(nf)
    key = (nf, masks.shape[0])
    if key not in _cache:
        _cache[key] = _Runner(_build(nf, masks.shape[0]))
    return _cache[key], masks


def make_in_maps(x, w_attn, w_proj, masks):

    xT = np.ascontiguousarray(x.T)
    aux = np.zeros((128, 416), np.float32)
    aux[:, 0:128] = np.eye(128, dtype=np.float32)
    aux[:, 128:160] = 1.0                  # vones
    aux[0, 160:224] = 1.0                  # bcast head0 pattern
    aux[0, 352:416] = 1.0                  # bcast head1 pattern

    in_maps = []
    for c in range(NCORES):
        h0, h1 = HPC * c, HPC * c + 1
        wq = np.concatenate([w_attn[:, D * h0:D * h0 + D],
                             w_attn[:, D * h1:D * h1 + D]], axis=1)
        wk = np.concatenate([w_attn[:, C + D * h0:C + D * h0 + D],
                             w_attn[:, C + D * h1:C + D * h1 + D]], axis=1)
        wv = np.concatenate([w_attn[:, 2 * C + D * h0:2 * C + D * h0 + D],
                             w_attn[:, 2 * C + D * h1:2 * C + D * h1 + D]], axis=1)
        wqkv = np.ascontiguousarray(np.concatenate([wq, wk, wv], axis=1))
        wp = np.ascontiguousarray(
            np.concatenate([w_proj[D * h0:D * h0 + D, :],
                            w_proj[D * h1:D * h1 + D, :]], axis=0))
        in_maps.append({
            "xT": xT, "wqkv": wqkv, "wp": wp,
            "masks": masks, "aux": aux,
        })
    return in_maps


def kernel(x, w_attn, w_proj, num_frames):
    x = np.asarray(x, dtype=np.float32)
    w_attn = np.asarray(w_attn, dtype=np.float32)
    w_proj = np.asarray(w_proj, dtype=np.float32)

    runner, masks = get_runner(num_frames)
    in_maps = make_in_maps(x, w_attn, w_proj, masks)
    outs = runner.run(runner.device_inputs(in_maps))
    results = runner.gather(outs)
    acc = np.zeros((T, C), np.float64)
    for rmap in results:
        acc += rmap["y_part"].astype(np.float64)
    return acc.astype(np.float32)
